# revision 44
# baseline (speedup 1.0000x reference)
"""Multi-head attention (RoPE + RMS-norm + structured mask bias) on 8 trn2
NeuronCores.

Sharding: B=4 batches x 2 half-head groups -> 8 cores. Core c handles batch
c//2 and heads 3*(c%2) .. 3*(c%2)+2. Each core computes per-head partial
outputs y_h @ Wproj_h.T summed over its 3 heads; the host adds the two
half-head partials per batch.

Math notes:
- bias = clip(g1*mi + g2*mj + g3*mi*mj, +-2) with mi,mj in {0,1} takes 4
  values; decomposed as a0 + a1*mi + a2*mj + a3*mi*mj. The a0 + a1*mi part
  is constant along the softmax axis (keys) and cancels; only
  (a2 + a3*mi)*mj survives. It is folded into the QK^T matmul as one extra
  contraction dim: q~ = [q*r/8 | a2+a3*mi], k~ = [k*r | mj].
- scores are in [-9.5, 9.5] (RMS-normed q,k => |q.k|<=64, /8, bias<=1.5),
  so softmax runs without max subtraction: exp, then one division by the
  sum. The denominator comes for free from an appended ones-column on v.
- All attention matmuls run in bf16 (fast weight load + 131ns/MM at N=512
  vs ~390ns for f32r); PSUM accumulation stays f32.
- rms scale = exp(-0.5*ln(ss+64eps)) so the whole kernel uses ONE ACT
  table set (natural_log_exp_and_others) -> one ~2.7us table load.
- exp of the scores is the per-core floor (~1 elem/cycle/lane on ScalarE
  only). Half the [128,1024] tiles run exact on ScalarE; the other half
  run on the otherwise-idle DVE as a Schraudolph-style approximation:
  bf16_bits = round(128*log2e*x + (16256 - 5.6)) written as int16 into
  the bf16 attention tile (|elem err| <= ~3.3%, final rel err ~1e-2
  vs the 2e-2 gate, verified in simulation).
- Softmax normalization: py (PSUM) is copied to SBUF immediately so the
  PSUM bank frees for the next head; reciprocal + gpsimd
  partition_broadcast + multiply run overlapped with the next head.
"""

from contextlib import ExitStack

import numpy as np
import ml_dtypes

import concourse.bass as bass
import concourse.tile as tile
from concourse import mybir
from concourse.masks import make_identity
from concourse.bass_utils import run_bass_kernel_spmd
from concourse.vector_clock import ScopedClock
import concourse.tile as tile_mod

B, T, C, H, D = 4, 2048, 384, 6, 64
NHC = 3           # heads per core
TCN = T // 128    # 16 token chunks
EPS = float(np.finfo(np.float32).eps)

# Schraudolph exp in bf16 bit space: bits = round(A*x + B)
LOG2E = 1.4426950408889634
A_EXP = 128.0 * LOG2E
B_EXP = 127.0 * 128.0 - 5.6

f32 = mybir.dt.float32
bf16 = mybir.dt.bfloat16
i16 = mybir.dt.int16
i32 = mybir.dt.int32
OP = mybir.AluOpType
AF = mybir.ActivationFunctionType
AX = mybir.AxisListType

# ---------------------------------------------------------------------------
# Workaround: this container's walrus accepts at most ONE sync wait per
# instruction. Split the TileContext final drain, and post-process all
# instructions, hoisting extra waits onto same-engine NoOps.
# ---------------------------------------------------------------------------
_ctr = [0]


def _drain_and_barrier_split(self, tick_clock, wait_clock):
    nc = self.nc
    drain_inst = nc.sync.drain()
    wait_clock.add_sem_waits(
        drain_inst.ins, ScopedClock({None: tick_clock.global_clock})
    )
    mi = drain_inst.ins
    si = mi.sync_info
    if si is not None and len(si.on_wait) > 1:
        waits = list(si.on_wait)
        mi.sync_info = mybir.SyncInfo(on_wait=waits[:1], on_update=list(si.on_update))
        for i in range(1, len(waits)):
            extra = nc.sync.drain()
            extra.ins.sync_info = mybir.SyncInfo(on_wait=[waits[i]], on_update=[])
    nc.all_engine_barrier()
    assert self.sems is not None
    popped = nc._tile_sem_poison_stack.pop()
    assert popped is self._sem_poison
    nc.clear_and_free_semaphores(list(self.sems.allocated().values()))
    nc.all_engine_barrier()


tile_mod.TileContext._drain_and_barrier = _drain_and_barrier_split


def _split_multi_waits(nc):
    for f in nc.m.functions:
        for bb in f.blocks:
            insts = bb.instructions
            out = []
            changed = False
            for inst in insts:
                si = getattr(inst, "sync_info", None)
                if si is not None and si.on_wait and len(si.on_wait) > 1:
                    waits = list(si.on_wait)
                    for w in waits[:-1]:
                        _ctr[0] += 1
                        out.append(
                            mybir.InstNoOp(
                                name=f"WSPLIT-{_ctr[0]}",
                                engine=inst.engine,
                                ins=[],
                                outs=[],
                                sync_info=mybir.SyncInfo(on_wait=[w], on_update=[]),
                            )
                        )
                    inst.sync_info = mybir.SyncInfo(
                        on_wait=[waits[-1]], on_update=list(si.on_update)
                    )
                    changed = True
                out.append(inst)
            if changed:
                insts[:] = out


# ---------------------------------------------------------------------------
# Device program (SPMD: same program on all 8 cores, data differs)
# ---------------------------------------------------------------------------


def _build_nc():
    nc = bass.Bass()
    # all tables partition-major so every DMA line is one contiguous burst
    xT = nc.declare_dram_parameter("xT", [C, T], bf16, isOutput=False)
    wqkv = nc.declare_dram_parameter("wqkv", [C, 576], bf16, isOutput=False)
    wpt = nc.declare_dram_parameter("wpt", [NHC, D, C], bf16, isOutput=False)
    # rope table: [...] = [cos | sin | -sin] packed per token chunk
    ropet = nc.declare_dram_parameter("ropet", [128, TCN, 3, 32], f32,
                                      isOutput=False)
    mtok = nc.declare_dram_parameter("mtok", [128, TCN], i32, isOutput=False)
    coef = nc.declare_dram_parameter("coef", [128, 2 * NHC], f32, isOutput=False)
    out = nc.declare_dram_parameter("out", [T, C], f32, isOutput=True)

    with tile.TileContext(nc) as tc, ExitStack() as ctx:
        const = ctx.enter_context(tc.tile_pool(name="const", bufs=1))
        big = ctx.enter_context(tc.tile_pool(name="big", bufs=1))

        # ---- loads: small consts first so phase A's elementwise work can
        # start as soon as the first QKV chunk lands ----
        # wq + first xt chunks first: the QKV matmuls only need these
        xt = big.tile([128, 3, T], bf16, tag="xt")
        wq = big.tile([128, 3, 576], bf16, tag="wq")
        xTr = xT[:].rearrange("(n p) t -> p n t", p=128)
        nc.sync.dma_start(out=wq[:], in_=wqkv[:].rearrange("(n p) d -> p n d", p=128))
        for cc in range(3):
            nc.sync.dma_start(out=xt[:, cc, 0:512], in_=xTr[:, cc, 0:512])
        rtb = const.tile([128, TCN, 3, 32], f32, tag="rtb")
        nc.sync.dma_start(out=rtb[:], in_=ropet[:])
        cs = rtb[:, :, 0, :]                     # [128, TCN, 32]
        sn2 = rtb[:, :, 1:3, :]                  # [128, TCN, 2, 32]
        mi_t = const.tile([128, TCN], i32, tag="mi")
        nc.sync.dma_start(out=mi_t[:], in_=mtok[:])
        cf = const.tile([128, 2 * NHC], f32, tag="cf")
        nc.sync.dma_start(out=cf[:], in_=coef[:])
        for cc in range(3):
            nc.sync.dma_start(out=xt[:, cc, 512:T], in_=xTr[:, cc, 512:T])
        wp = const.tile([D, NHC, C], bf16, tag="wp")
        nc.sync.dma_start(out=wp[:], in_=wpt[:].rearrange("h d c -> d h c"))

        ident = const.tile([128, 128], bf16, tag="ident")
        make_identity(nc, ident[:])
        mf = const.tile([128, TCN], f32, tag="mf")
        nc.vector.tensor_copy(out=mf[:], in_=mi_t[:])
        mfb = const.tile([128, TCN], bf16, tag="mfb")
        nc.gpsimd.tensor_copy(out=mfb[:], in_=mf[:])
        onesb = const.tile([128, TCN], bf16, tag="onesb")
        nc.vector.memset(onesb[:], 1.0)
        eps64_t = const.tile([128, 1], f32, tag="eps64")
        nc.vector.memset(eps64_t[:], EPS * D)

        # v-augment coefficients per (token-chunk, head): a2 + a3*m
        vcol = const.tile([128, TCN, NHC], bf16, tag="vcol")
        for h in range(NHC):
            nc.vector.tensor_scalar(
                out=vcol[:, :, h], in0=mf[:],
                scalar1=cf[:, 2 * h + 1 : 2 * h + 2],
                scalar2=cf[:, 2 * h : 2 * h + 1],
                op0=OP.mult, op1=OP.add,
            )

        # persistent tensors: qkt[d, qk, h, t] (q~ and k~, transposed);
        # vsb[t-part, chunk, h, d+1]; yn[h] normalized attention output
        qkt = big.tile([65, 2, NHC, T], bf16, tag="qkt")
        vsb = big.tile([128, TCN, NHC, 65], bf16, tag="vsb")
        # yn rows 0-63: unnormalized head output; row 64: softmax denominator
        yn = [big.tile([65, T], bf16, name=f"yn{h}", tag=f"yn{h}")
              for h in range(NHC)]
        nc.vector.tensor_copy(
            out=vsb[:, :, :, 64], in_=onesb[:, :, None].to_broadcast([128, TCN, NHC])
        )

        # ---- Phase A: QKV projection, rope, rms, augment, transpose ----
        # Phase A is software-pipelined: each engine only sees work whose
        # inputs are 1+ iterations old, so FIFO queues never head-of-line
        # block on the current iteration's cross-engine chain.
        with tc.tile_pool(name="psA", bufs=3, space="PSUM") as psA, \
             tc.tile_pool(name="psT", bufs=2, space="PSUM") as psT, \
             tc.tile_pool(name="scrA", bufs=6) as scrA:
            pq_l, ro_l, rt_l, rr_l, aug_l = {}, {}, {}, {}, {}

            def stage0(t):   # QKV projection (tensor)
                pq = psA.tile([128, 576], f32, tag="pq")
                pq_l[t] = pq
                tsl = slice(t * 128, (t + 1) * 128)
                for cc in range(3):
                    lhs = xt[:, cc, tsl]
                    nc.tensor.matmul(
                        pq[:, 0:512], lhs, wq[:, cc, 0:512],
                        start=(cc == 0), stop=(cc == 2),
                    )
                    nc.tensor.matmul(
                        pq[:, 512:576], lhs, wq[:, cc, 512:576],
                        start=(cc == 0), stop=(cc == 2),
                    )

            def stage1(t):   # rope mults (DVE) + v copy (scalar); frees pq
                pq = pq_l.pop(t)
                z5 = pq[:, 0:384].rearrange("p (hq hf d) -> p hq hf d",
                                            hq=6, hf=2)
                csb = cs[:, t, None, None, :].to_broadcast([128, 6, 2, 32])
                snb = sn2[:, t, None, :, :].to_broadcast([128, 6, 2, 32])
                ro = scrA.tile([128, 6, 2, 32], f32, tag="ro")
                rt = scrA.tile([128, 6, 2, 32], f32, tag="rt")
                ro_l[t], rt_l[t] = ro, rt
                nc.vector.tensor_tensor(ro[:], z5, csb, OP.mult)
                zswap = bass.AP(
                    tensor=pq.tensor, offset=pq.offset + 32,
                    ap=[list(pq.ap[0])] + [[64, 6], [-32, 2], [1, 32]],
                )
                nc.vector.tensor_tensor(rt[:], zswap, snb, OP.mult)
                nc.scalar.activation(
                    out=vsb[:, t, :, 0:64],
                    in_=pq[:, 384:576].rearrange("p (h d) -> p h d", h=NHC),
                    func=AF.Copy,
                )

            def stage2(t):   # rope add (gps), square (DVE), reduce (DVE),
                pq = None    # rms scale = exp(-0.5*ln(ss+64eps)) (scalar)
                ro, rt = ro_l[t], rt_l.pop(t)
                nc.gpsimd.tensor_tensor(ro[:], ro[:], rt[:], OP.add)
                ro4 = ro[:].rearrange("p (h qk) hf d -> p h qk (hf d)", h=NHC)
                sq = scrA.tile([128, NHC, 2, D], bf16, tag="sq")
                nc.vector.tensor_tensor(sq[:], ro4, ro4, OP.mult)
                ssum = scrA.tile([128, NHC, 2], f32, tag="ssum")
                nc.vector.tensor_reduce(out=ssum[:], in_=sq[:], axis=AX.X,
                                        op=OP.add)
                lnv = scrA.tile([128, NHC, 2], f32, tag="lnv")
                nc.scalar.activation(
                    out=lnv[:], in_=ssum[:], func=AF.Ln, bias=eps64_t[:],
                )
                rr = scrA.tile([128, NHC, 2], f32, tag="rr")
                rr_l[t] = rr
                nc.scalar.activation(out=rr[:], in_=lnv[:], func=AF.Exp,
                                     scale=-0.5)

            def stage3(t):   # k-scale + augment (gps)
                ro = ro_l.pop(t)
                rr = rr_l.pop(t)
                ro4 = ro[:].rearrange("p (h qk) hf d -> p h qk (hf d)", h=NHC)
                nc.gpsimd.tensor_scalar_mul(out=rr[:, :, 1], in0=rr[:, :, 1],
                                            scalar1=8.0)
                aug = scrA.tile([128, NHC, 2, 65], bf16, tag="aug")
                aug_l[t] = aug
                nc.gpsimd.tensor_copy(out=aug[:, :, 0, 64], in_=vcol[:, t, :])
                nc.gpsimd.tensor_copy(
                    out=aug[:, :, 1, 64],
                    in_=mfb[:, t : t + 1].to_broadcast([128, NHC]),
                )
                nc.gpsimd.tensor_tensor(
                    aug[:, :, :, 0:64], ro4,
                    rr[:, :, :, None].to_broadcast([128, NHC, 2, D]), OP.mult,
                )

            def stage4(t):   # transposes (tensor) + qkt copies (scalar/DVE)
                aug = aug_l.pop(t)
                tsl = slice(t * 128, (t + 1) * 128)
                ptr = psT.tile([65, 2, NHC * 128], bf16, tag="pt")
                for qk in range(2):
                    for h in range(NHC):
                        nc.tensor.transpose(
                            out=ptr[:, qk, h * 128 : (h + 1) * 128],
                            in_=aug[:, h, qk, :], identity=ident[:],
                        )
                    src = ptr[:, qk, :].rearrange("d (h c) -> d h c", h=NHC)
                    if qk == 0:
                        nc.scalar.activation(
                            out=qkt[:, 0, :, tsl], in_=src, func=AF.Copy,
                        )
                    else:
                        nc.vector.tensor_copy(out=qkt[:, 1, :, tsl], in_=src)

            stages = [stage0, stage1, stage2, stage3, stage4]
            for i in range(TCN + 4):
                for s, fn in enumerate(stages):
                    t = i - s
                    if 0 <= t < TCN:
                        fn(t)

        # ---- Phase B: attention (scores -> exp -> PV), normalize ----
        with tc.tile_pool(name="psS", bufs=2, space="PSUM") as psS, \
             tc.tile_pool(name="psY", bufs=1, space="PSUM") as psY, \
             tc.tile_pool(name="att", bufs=6) as attp:
            for h in range(NHC):
                py = psY.tile([65, T], f32, tag="py")
                ats = [None] * TCN

                def emit_pv(j, h=h, py=py, ats=ats):
                    for n in range(4):
                        nc.tensor.matmul(
                            py[:, n * 512 : (n + 1) * 512], vsb[:, j, h, :],
                            ats[j][:, n * 512 : (n + 1) * 512],
                            start=(j == 0), stop=(j == TCN - 1),
                        )

                for j in range(TCN):
                    kblk = qkt[:, 1, h, j * 128 : (j + 1) * 128]
                    at = attp.tile([128, T], bf16, tag="at")
                    ats[j] = at
                    for half in range(2):
                        ps = psS.tile([128, 1024], f32, tag="ps")
                        for n2 in range(2):
                            n = half * 2 + n2
                            nc.tensor.matmul(
                                ps[:, n2 * 512 : (n2 + 1) * 512], kblk,
                                qkt[:, 0, h, n * 512 : (n + 1) * 512],
                                start=True, stop=True,
                            )
                        asl = slice(half * 1024, (half + 1) * 1024)
                        if (2 * j + half) % 8 not in (1, 3, 6):
                            # exact exp on ScalarE
                            nc.scalar.activation(
                                out=at[:, asl], in_=ps[:], func=AF.Exp,
                            )
                        else:
                            # Schraudolph exp on DVE: bf16 bits via int16
                            nc.vector.tensor_scalar(
                                out=at[:, asl].bitcast(i16), in0=ps[:],
                                scalar1=A_EXP, scalar2=B_EXP,
                                op0=OP.mult, op1=OP.add,
                            )
                    if j >= 1:
                        emit_pv(j - 1)
                emit_pv(TCN - 1)
                # single copy frees the PSUM banks; division deferred to C
                # (on ScalarE: it has headroom and reads PSUM fast)
                nc.scalar.activation(out=yn[h][:], in_=py[:], func=AF.Copy)

        # ---- Phase C: per-head projection, then combine with 1/den ----
        # Denominators (row 64 of each yn) are moved to token-partition
        # layout with tiny PE transposes, reciprocal'd once on 128 lanes,
        # then applied as per-partition scales while summing heads.
        with tc.tile_pool(name="psC", bufs=2, space="PSUM") as psC, \
             tc.tile_pool(name="psD", bufs=1, space="PSUM") as psD, \
             tc.tile_pool(name="outp", bufs=3) as outp:
            # bf16 PSUM writes must be 4B aligned -> pad a dummy lane per den
            dt_ps = psD.tile([128, TCN, NHC, 2], bf16, tag="dt")
            for t in range(TCN):
                for h in range(NHC):
                    nc.tensor.transpose(
                        out=dt_ps[:, t, h, 0, None],
                        in_=yn[h][64:65, t * 128 : (t + 1) * 128],
                        identity=ident[64:65, 64:65],
                    )
            rcp = const.tile([128, TCN, NHC], f32, tag="rcp")
            nc.vector.reciprocal(out=rcp[:], in_=dt_ps[:, :, :, 0])
            for tg in range(TCN // 4):
                ob = outp.tile([128, 4, C], f32, tag="ob")
                for tt in range(4):
                    t = tg * 4 + tt
                    tsl = slice(t * 128, (t + 1) * 128)
                    po = [psC.tile([128, C], f32, name=f"po{h}", tag=f"po{h}")
                          for h in range(NHC)]
                    for h in range(NHC):
                        nc.tensor.matmul(
                            po[h][:], yn[h][0:64, tsl], wp[:, h, :],
                            start=True, stop=True,
                        )
                    acc = outp.tile([128, C], f32, tag="acc")
                    nc.scalar.activation(
                        out=acc[:], in_=po[0][:], func=AF.Copy,
                        scale=rcp[:, t, 0, None],
                    )
                    nc.vector.scalar_tensor_tensor(
                        out=acc[:], in0=po[1][:], scalar=rcp[:, t, 1, None],
                        in1=acc[:], op0=OP.mult, op1=OP.add,
                    )
                    nc.vector.scalar_tensor_tensor(
                        out=ob[:, tt, :], in0=po[2][:], scalar=rcp[:, t, 2, None],
                        in1=acc[:], op0=OP.mult, op1=OP.add,
                    )
                nc.sync.dma_start(
                    out=out[tg * 512 : (tg + 1) * 512, :].rearrange(
                        "(n p) c -> p n c", p=128),
                    in_=ob[:],
                )

    _split_multi_waits(nc)
    return nc


_NC = None
LAST_RESULTS = None


def _get_nc():
    global _NC
    if _NC is None:
        _NC = _build_nc()
    return _NC


def kernel(x, cos, sin, token_is_mask, Wq, Wk, Wv, Wproj, mask_bias_raw,
           bias_scale, **_kw):
    bf = ml_dtypes.bfloat16
    x = np.asarray(x, np.float32)
    cos2 = np.asarray(cos, np.float32)[0, :, 0, :]                         # (T,32)
    sin2 = np.asarray(sin, np.float32)[0, :, 0, :]
    # partition-major rope table [128, TCN, 3, 32] = [cos | sin | -sin],
    # token t = n*128 + p
    rt3 = np.stack([cos2, sin2, -sin2], axis=1)                            # (T,3,32)
    ropet = np.ascontiguousarray(
        rt3.reshape(TCN, 128, 3, 32).transpose(1, 0, 2, 3))
    m = np.asarray(token_is_mask, np.int32)
    Wq = np.asarray(Wq, np.float32)
    Wk = np.asarray(Wk, np.float32)
    Wv = np.asarray(Wv, np.float32)
    Wp = np.asarray(Wproj, np.float32)
    g = (0.5 * np.tanh(np.asarray(mask_bias_raw, np.float64))
         * float(np.asarray(bias_scale))).astype(np.float32)  # (H,3)

    in_maps = []
    for core in range(8):
        b = core // 2
        hs = NHC * (core % 2)
        xTb = np.ascontiguousarray(x[b].T).astype(bf)          # (C,T)
        wqkv = np.zeros((C, 576), np.float32)
        wpt = np.zeros((NHC, D, C), np.float32)
        coefs = np.zeros((2 * NHC,), np.float32)
        for i in range(NHC):
            h = hs + i
            sl = slice(h * D, (h + 1) * D)
            wqkv[:, i * 128 + 0 : i * 128 + 64] = Wq[sl].T
            wqkv[:, i * 128 + 64 : i * 128 + 128] = Wk[sl].T
            wqkv[:, 384 + i * 64 : 384 + (i + 1) * 64] = Wv[sl].T
            wpt[i] = Wp[:, sl].T
            b01 = float(np.clip(g[h, 1], -2.0, 2.0))
            b10 = float(np.clip(g[h, 0], -2.0, 2.0))
            b11 = float(np.clip(g[h, 0] + g[h, 1] + g[h, 2], -2.0, 2.0))
            coefs[2 * i] = b01            # a2
            coefs[2 * i + 1] = b11 - b10 - b01  # a3
        in_maps.append(
            dict(
                xT=xTb,
                wqkv=wqkv.astype(bf),
                wpt=wpt.astype(bf),
                ropet=ropet,
                mtok=np.ascontiguousarray(m[b].reshape(TCN, 128).T),
                coef=np.tile(coefs[None, :], (128, 1)),
            )
        )

    nc = _get_nc()
    res = run_bass_kernel_spmd(nc, in_maps, list(range(8)))
    global LAST_RESULTS
    LAST_RESULTS = res
    out = np.zeros((B, T, C), np.float32)
    for b in range(B):
        out[b] = res.results[2 * b]["out"] + res.results[2 * b + 1]["out"]
    return out


# revision 47
# speedup vs baseline: 1.0533x; 1.0533x over previous
"""Multi-head attention (RoPE + RMS-norm + structured mask bias) on 8 trn2
NeuronCores.

Sharding: B=4 batches x 2 half-head groups -> 8 cores. Core c handles batch
c//2 and heads 3*(c%2) .. 3*(c%2)+2. Each core computes per-head partial
outputs y_h @ Wproj_h.T summed over its 3 heads; the host adds the two
half-head partials per batch.

Math notes:
- bias = clip(g1*mi + g2*mj + g3*mi*mj, +-2) with mi,mj in {0,1} takes 4
  values; decomposed as a0 + a1*mi + a2*mj + a3*mi*mj. The a0 + a1*mi part
  is constant along the softmax axis (keys) and cancels; only
  (a2 + a3*mi)*mj survives. It is folded into the QK^T matmul as one extra
  contraction dim: q~ = [q*r/8 | a2+a3*mi], k~ = [k*r | mj].
- scores are in [-9.5, 9.5] (RMS-normed q,k => |q.k|<=64, /8, bias<=1.5),
  so softmax runs without max subtraction: exp, then one division by the
  sum. The denominator comes for free from an appended ones-column on v.
- All attention matmuls run in bf16 (fast weight load + 131ns/MM at N=512
  vs ~390ns for f32r); PSUM accumulation stays f32.
- rms scale = exp(-0.5*ln(ss+64eps)) so the whole kernel uses ONE ACT
  table set (natural_log_exp_and_others) -> one ~2.7us table load.
- exp of the scores is the per-core floor (~1 elem/cycle/lane on ScalarE
  only). Half the [128,1024] tiles run exact on ScalarE; the other half
  run on the otherwise-idle DVE as a Schraudolph-style approximation:
  bf16_bits = round(128*log2e*x + (16256 - 5.6)) written as int16 into
  the bf16 attention tile (|elem err| <= ~3.3%, final rel err ~1e-2
  vs the 2e-2 gate, verified in simulation).
- Softmax normalization: py (PSUM) is copied to SBUF immediately so the
  PSUM bank frees for the next head; reciprocal + gpsimd
  partition_broadcast + multiply run overlapped with the next head.
"""

from contextlib import ExitStack

import numpy as np
import ml_dtypes

import concourse.bass as bass
import concourse.tile as tile
from concourse import mybir
from concourse.masks import make_identity
from concourse.bass_utils import run_bass_kernel_spmd
from concourse.vector_clock import ScopedClock
import concourse.tile as tile_mod

B, T, C, H, D = 4, 2048, 384, 6, 64
NHC = 3           # heads per core
TCN = T // 128    # 16 token chunks
EPS = float(np.finfo(np.float32).eps)

# Schraudolph exp in bf16 bit space: bits = round(A*x + B)
LOG2E = 1.4426950408889634
A_EXP = 128.0 * LOG2E
B_EXP = 127.0 * 128.0 - 5.6

f32 = mybir.dt.float32
bf16 = mybir.dt.bfloat16
i16 = mybir.dt.int16
i32 = mybir.dt.int32
OP = mybir.AluOpType
AF = mybir.ActivationFunctionType
AX = mybir.AxisListType

# ---------------------------------------------------------------------------
# Workaround: this container's walrus accepts at most ONE sync wait per
# instruction. Split the TileContext final drain, and post-process all
# instructions, hoisting extra waits onto same-engine NoOps.
# ---------------------------------------------------------------------------
_ctr = [0]


def _drain_and_barrier_split(self, tick_clock, wait_clock):
    nc = self.nc
    drain_inst = nc.sync.drain()
    wait_clock.add_sem_waits(
        drain_inst.ins, ScopedClock({None: tick_clock.global_clock})
    )
    mi = drain_inst.ins
    si = mi.sync_info
    if si is not None and len(si.on_wait) > 1:
        waits = list(si.on_wait)
        mi.sync_info = mybir.SyncInfo(on_wait=waits[:1], on_update=list(si.on_update))
        for i in range(1, len(waits)):
            extra = nc.sync.drain()
            extra.ins.sync_info = mybir.SyncInfo(on_wait=[waits[i]], on_update=[])
    nc.all_engine_barrier()
    assert self.sems is not None
    popped = nc._tile_sem_poison_stack.pop()
    assert popped is self._sem_poison
    nc.clear_and_free_semaphores(list(self.sems.allocated().values()))
    nc.all_engine_barrier()


tile_mod.TileContext._drain_and_barrier = _drain_and_barrier_split


def _split_multi_waits(nc):
    for f in nc.m.functions:
        for bb in f.blocks:
            insts = bb.instructions
            out = []
            changed = False
            for inst in insts:
                si = getattr(inst, "sync_info", None)
                if si is not None and si.on_wait and len(si.on_wait) > 1:
                    waits = list(si.on_wait)
                    for w in waits[:-1]:
                        _ctr[0] += 1
                        out.append(
                            mybir.InstNoOp(
                                name=f"WSPLIT-{_ctr[0]}",
                                engine=inst.engine,
                                ins=[],
                                outs=[],
                                sync_info=mybir.SyncInfo(on_wait=[w], on_update=[]),
                            )
                        )
                    inst.sync_info = mybir.SyncInfo(
                        on_wait=[waits[-1]], on_update=list(si.on_update)
                    )
                    changed = True
                out.append(inst)
            if changed:
                insts[:] = out


# ---------------------------------------------------------------------------
# Device program (SPMD: same program on all 8 cores, data differs)
# ---------------------------------------------------------------------------


def _build_nc():
    nc = bass.Bass()
    # all tables partition-major so every DMA line is one contiguous burst
    xT = nc.declare_dram_parameter("xT", [C, T], bf16, isOutput=False)
    wqkv = nc.declare_dram_parameter("wqkv", [C, 576], bf16, isOutput=False)
    wpt = nc.declare_dram_parameter("wpt", [NHC, D, C], bf16, isOutput=False)
    # rope table: [...] = [cos | sin | -sin] packed per token chunk
    ropet = nc.declare_dram_parameter("ropet", [128, TCN, 3, 32], f32,
                                      isOutput=False)
    mtok = nc.declare_dram_parameter("mtok", [128, TCN], i32, isOutput=False)
    coef = nc.declare_dram_parameter("coef", [128, 2 * NHC], f32, isOutput=False)
    out = nc.declare_dram_parameter("out", [T, C], f32, isOutput=True)

    with tile.TileContext(nc) as tc, ExitStack() as ctx:
        const = ctx.enter_context(tc.tile_pool(name="const", bufs=1))
        big = ctx.enter_context(tc.tile_pool(name="big", bufs=1))

        # ---- loads: small consts first so phase A's elementwise work can
        # start as soon as the first QKV chunk lands ----
        # wq + first xt chunks first: the QKV matmuls only need these
        xt = big.tile([128, 3, T], bf16, tag="xt")
        wq = big.tile([128, 3, 576], bf16, tag="wq")
        xTr = xT[:].rearrange("(n p) t -> p n t", p=128)
        nc.sync.dma_start(out=wq[:], in_=wqkv[:].rearrange("(n p) d -> p n d", p=128))
        for cc in range(3):
            nc.sync.dma_start(out=xt[:, cc, 0:512], in_=xTr[:, cc, 0:512])
        rtb = const.tile([128, TCN, 3, 32], f32, tag="rtb")
        nc.sync.dma_start(out=rtb[:], in_=ropet[:])
        cs = rtb[:, :, 0, :]                     # [128, TCN, 32]
        sn2 = rtb[:, :, 1:3, :]                  # [128, TCN, 2, 32]
        mi_t = const.tile([128, TCN], i32, tag="mi")
        nc.sync.dma_start(out=mi_t[:], in_=mtok[:])
        cf = const.tile([128, 2 * NHC], f32, tag="cf")
        nc.sync.dma_start(out=cf[:], in_=coef[:])
        for cc in range(3):
            nc.sync.dma_start(out=xt[:, cc, 512:T], in_=xTr[:, cc, 512:T])
        wp = const.tile([D, NHC, C], bf16, tag="wp")
        nc.sync.dma_start(out=wp[:], in_=wpt[:].rearrange("h d c -> d h c"))

        ident = const.tile([128, 128], bf16, tag="ident")
        make_identity(nc, ident[:])
        mf = const.tile([128, TCN], f32, tag="mf")
        nc.vector.tensor_copy(out=mf[:], in_=mi_t[:])
        mfb = const.tile([128, TCN], bf16, tag="mfb")
        nc.gpsimd.tensor_copy(out=mfb[:], in_=mf[:])
        onesb = const.tile([128, TCN], bf16, tag="onesb")
        nc.vector.memset(onesb[:], 1.0)
        eps64_t = const.tile([128, 1], f32, tag="eps64")
        nc.vector.memset(eps64_t[:], EPS * D)

        # v-augment coefficients per (token-chunk, head): a2 + a3*m
        vcol = const.tile([128, TCN, NHC], bf16, tag="vcol")
        for h in range(NHC):
            nc.vector.tensor_scalar(
                out=vcol[:, :, h], in0=mf[:],
                scalar1=cf[:, 2 * h + 1 : 2 * h + 2],
                scalar2=cf[:, 2 * h : 2 * h + 1],
                op0=OP.mult, op1=OP.add,
            )

        # persistent tensors: qkt[d, qk, h, t] (q~ and k~, transposed);
        # vsb[t-part, chunk, h, d+1]; yn[h] normalized attention output
        qkt = big.tile([65, 2, NHC, T], bf16, tag="qkt")
        vsb = big.tile([128, TCN, NHC, 65], bf16, tag="vsb")
        # yn rows 0-63: unnormalized head output; row 64: softmax denominator
        yn = [big.tile([65, T], bf16, name=f"yn{h}", tag=f"yn{h}")
              for h in range(NHC)]
        nc.vector.tensor_copy(
            out=vsb[:, :, :, 64], in_=onesb[:, :, None].to_broadcast([128, TCN, NHC])
        )

        # ---- Phase A: QKV projection, rope, rms, augment, transpose ----
        # Phase A is software-pipelined: each engine only sees work whose
        # inputs are 1+ iterations old, so FIFO queues never head-of-line
        # block on the current iteration's cross-engine chain.
        with tc.tile_pool(name="psA", bufs=3, space="PSUM") as psA, \
             tc.tile_pool(name="psT", bufs=2, space="PSUM") as psT, \
             tc.tile_pool(name="scrA", bufs=6) as scrA:
            pq_l, ro_l, rt_l, rr_l, aug_l = {}, {}, {}, {}, {}

            def stage0(t):   # QKV projection (tensor)
                pq = psA.tile([128, 576], f32, tag="pq")
                pq_l[t] = pq
                tsl = slice(t * 128, (t + 1) * 128)
                for cc in range(3):
                    lhs = xt[:, cc, tsl]
                    nc.tensor.matmul(
                        pq[:, 0:512], lhs, wq[:, cc, 0:512],
                        start=(cc == 0), stop=(cc == 2),
                    )
                    nc.tensor.matmul(
                        pq[:, 512:576], lhs, wq[:, cc, 512:576],
                        start=(cc == 0), stop=(cc == 2),
                    )

            def stage1(t):   # rope mults (DVE) + v copy (scalar); frees pq
                pq = pq_l.pop(t)
                z5 = pq[:, 0:384].rearrange("p (hq hf d) -> p hq hf d",
                                            hq=6, hf=2)
                csb = cs[:, t, None, None, :].to_broadcast([128, 6, 2, 32])
                snb = sn2[:, t, None, :, :].to_broadcast([128, 6, 2, 32])
                ro = scrA.tile([128, 6, 2, 32], f32, tag="ro")
                rt = scrA.tile([128, 6, 2, 32], f32, tag="rt")
                ro_l[t], rt_l[t] = ro, rt
                nc.vector.tensor_tensor(ro[:], z5, csb, OP.mult)
                zswap = bass.AP(
                    tensor=pq.tensor, offset=pq.offset + 32,
                    ap=[list(pq.ap[0])] + [[64, 6], [-32, 2], [1, 32]],
                )
                nc.vector.tensor_tensor(rt[:], zswap, snb, OP.mult)
                nc.scalar.activation(
                    out=vsb[:, t, :, 0:64],
                    in_=pq[:, 384:576].rearrange("p (h d) -> p h d", h=NHC),
                    func=AF.Copy,
                )

            def stage2(t):   # rope add (gps), square (DVE), reduce (DVE),
                pq = None    # rms scale = exp(-0.5*ln(ss+64eps)) (scalar)
                ro, rt = ro_l[t], rt_l.pop(t)
                nc.gpsimd.tensor_tensor(ro[:], ro[:], rt[:], OP.add)
                ro4 = ro[:].rearrange("p (h qk) hf d -> p h qk (hf d)", h=NHC)
                sq = scrA.tile([128, NHC, 2, D], bf16, tag="sq")
                nc.vector.tensor_tensor(sq[:], ro4, ro4, OP.mult)
                ssum = scrA.tile([128, NHC, 2], f32, tag="ssum")
                nc.vector.tensor_reduce(out=ssum[:], in_=sq[:], axis=AX.X,
                                        op=OP.add)
                lnv = scrA.tile([128, NHC, 2], f32, tag="lnv")
                nc.scalar.activation(
                    out=lnv[:], in_=ssum[:], func=AF.Ln, bias=eps64_t[:],
                )
                rr = scrA.tile([128, NHC, 2], f32, tag="rr")
                rr_l[t] = rr
                nc.scalar.activation(out=rr[:], in_=lnv[:], func=AF.Exp,
                                     scale=-0.5)

            def stage3(t):   # k-scale + augment (gps)
                ro = ro_l.pop(t)
                rr = rr_l.pop(t)
                ro4 = ro[:].rearrange("p (h qk) hf d -> p h qk (hf d)", h=NHC)
                nc.gpsimd.tensor_scalar_mul(out=rr[:, :, 1], in0=rr[:, :, 1],
                                            scalar1=8.0)
                aug = scrA.tile([128, NHC, 2, 65], bf16, tag="aug")
                aug_l[t] = aug
                nc.gpsimd.tensor_copy(out=aug[:, :, 0, 64], in_=vcol[:, t, :])
                nc.gpsimd.tensor_copy(
                    out=aug[:, :, 1, 64],
                    in_=mfb[:, t : t + 1].to_broadcast([128, NHC]),
                )
                nc.gpsimd.tensor_tensor(
                    aug[:, :, :, 0:64], ro4,
                    rr[:, :, :, None].to_broadcast([128, NHC, 2, D]), OP.mult,
                )

            def stage4(t):   # transposes (tensor) + qkt copies (scalar/DVE)
                aug = aug_l.pop(t)
                tsl = slice(t * 128, (t + 1) * 128)
                ptr = psT.tile([65, 2, NHC * 128], bf16, tag="pt")
                for qk in range(2):
                    for h in range(NHC):
                        nc.tensor.transpose(
                            out=ptr[:, qk, h * 128 : (h + 1) * 128],
                            in_=aug[:, h, qk, :], identity=ident[:],
                        )
                    src = ptr[:, qk, :].rearrange("d (h c) -> d h c", h=NHC)
                    if qk == 0:
                        nc.scalar.activation(
                            out=qkt[:, 0, :, tsl], in_=src, func=AF.Copy,
                        )
                    else:
                        nc.vector.tensor_copy(out=qkt[:, 1, :, tsl], in_=src)

            skew = [(stage0, 0), (stage1, 1), (stage2, 2), (stage3, 3),
                    (stage4, 6)]
            for i in range(TCN + 6):
                for fn, dist in skew:
                    t = i - dist
                    if 0 <= t < TCN:
                        fn(t)

        # ---- Phase B: attention (scores -> exp -> PV), normalize ----
        with tc.tile_pool(name="psS", bufs=2, space="PSUM") as psS, \
             tc.tile_pool(name="psY", bufs=1, space="PSUM") as psY, \
             tc.tile_pool(name="att", bufs=6) as attp:
            for h in range(NHC):
                py = psY.tile([65, T], f32, tag="py")
                ats = [None] * TCN

                def emit_pv(j, h=h, py=py, ats=ats):
                    for n in range(4):
                        nc.tensor.matmul(
                            py[:, n * 512 : (n + 1) * 512], vsb[:, j, h, :],
                            ats[j][:, n * 512 : (n + 1) * 512],
                            start=(j == 0), stop=(j == TCN - 1),
                        )

                for j in range(TCN):
                    kblk = qkt[:, 1, h, j * 128 : (j + 1) * 128]
                    at = attp.tile([128, T], bf16, tag="at")
                    ats[j] = at
                    for half in range(2):
                        ps = psS.tile([128, 1024], f32, tag="ps")
                        for n2 in range(2):
                            n = half * 2 + n2
                            nc.tensor.matmul(
                                ps[:, n2 * 512 : (n2 + 1) * 512], kblk,
                                qkt[:, 0, h, n * 512 : (n + 1) * 512],
                                start=True, stop=True,
                            )
                        asl = slice(half * 1024, (half + 1) * 1024)
                        if (2 * j + half) % 4 != 1:
                            # exact exp on ScalarE
                            nc.scalar.activation(
                                out=at[:, asl], in_=ps[:], func=AF.Exp,
                            )
                        else:
                            # Schraudolph exp on DVE: bf16 bits via int16
                            nc.vector.tensor_scalar(
                                out=at[:, asl].bitcast(i16), in0=ps[:],
                                scalar1=A_EXP, scalar2=B_EXP,
                                op0=OP.mult, op1=OP.add,
                            )
                    if j >= 1:
                        emit_pv(j - 1)
                emit_pv(TCN - 1)
                # single copy frees the PSUM banks; division deferred to C
                # (on ScalarE: it has headroom and reads PSUM fast)
                nc.scalar.activation(out=yn[h][:], in_=py[:], func=AF.Copy)

        # ---- Phase C: per-head projection, then combine with 1/den ----
        # Denominators (row 64 of each yn) are moved to token-partition
        # layout with tiny PE transposes, reciprocal'd once on 128 lanes,
        # then applied as per-partition scales while summing heads.
        with tc.tile_pool(name="psC", bufs=2, space="PSUM") as psC, \
             tc.tile_pool(name="psD", bufs=1, space="PSUM") as psD, \
             tc.tile_pool(name="outp", bufs=3) as outp:
            # bf16 PSUM writes must be 4B aligned -> pad a dummy lane per den
            dt_ps = psD.tile([128, TCN, NHC, 2], bf16, tag="dt")
            for t in range(TCN):
                for h in range(NHC):
                    nc.tensor.transpose(
                        out=dt_ps[:, t, h, 0, None],
                        in_=yn[h][64:65, t * 128 : (t + 1) * 128],
                        identity=ident[64:65, 64:65],
                    )
            rcp = const.tile([128, TCN, NHC], f32, tag="rcp")
            nc.vector.reciprocal(out=rcp[:], in_=dt_ps[:, :, :, 0])
            for tg in range(TCN // 4):
                ob = outp.tile([128, 4, C], f32, tag="ob")
                for tt in range(4):
                    t = tg * 4 + tt
                    tsl = slice(t * 128, (t + 1) * 128)
                    po = [psC.tile([128, C], f32, name=f"po{h}", tag=f"po{h}")
                          for h in range(NHC)]
                    for h in range(NHC):
                        nc.tensor.matmul(
                            po[h][:], yn[h][0:64, tsl], wp[:, h, :],
                            start=True, stop=True,
                        )
                    acc = outp.tile([128, C], f32, tag="acc")
                    nc.scalar.activation(
                        out=acc[:], in_=po[0][:], func=AF.Copy,
                        scale=rcp[:, t, 0, None],
                    )
                    nc.vector.scalar_tensor_tensor(
                        out=acc[:], in0=po[1][:], scalar=rcp[:, t, 1, None],
                        in1=acc[:], op0=OP.mult, op1=OP.add,
                    )
                    nc.vector.scalar_tensor_tensor(
                        out=ob[:, tt, :], in0=po[2][:], scalar=rcp[:, t, 2, None],
                        in1=acc[:], op0=OP.mult, op1=OP.add,
                    )
                nc.sync.dma_start(
                    out=out[tg * 512 : (tg + 1) * 512, :].rearrange(
                        "(n p) c -> p n c", p=128),
                    in_=ob[:],
                )

    _split_multi_waits(nc)
    return nc


_NC = None
LAST_RESULTS = None


def _get_nc():
    global _NC
    if _NC is None:
        _NC = _build_nc()
    return _NC


def kernel(x, cos, sin, token_is_mask, Wq, Wk, Wv, Wproj, mask_bias_raw,
           bias_scale, **_kw):
    bf = ml_dtypes.bfloat16
    x = np.asarray(x, np.float32)
    cos2 = np.asarray(cos, np.float32)[0, :, 0, :]                         # (T,32)
    sin2 = np.asarray(sin, np.float32)[0, :, 0, :]
    # partition-major rope table [128, TCN, 3, 32] = [cos | sin | -sin],
    # token t = n*128 + p
    rt3 = np.stack([cos2, sin2, -sin2], axis=1)                            # (T,3,32)
    ropet = np.ascontiguousarray(
        rt3.reshape(TCN, 128, 3, 32).transpose(1, 0, 2, 3))
    m = np.asarray(token_is_mask, np.int32)
    Wq = np.asarray(Wq, np.float32)
    Wk = np.asarray(Wk, np.float32)
    Wv = np.asarray(Wv, np.float32)
    Wp = np.asarray(Wproj, np.float32)
    g = (0.5 * np.tanh(np.asarray(mask_bias_raw, np.float64))
         * float(np.asarray(bias_scale))).astype(np.float32)  # (H,3)

    in_maps = []
    for core in range(8):
        b = core // 2
        hs = NHC * (core % 2)
        xTb = np.ascontiguousarray(x[b].T).astype(bf)          # (C,T)
        wqkv = np.zeros((C, 576), np.float32)
        wpt = np.zeros((NHC, D, C), np.float32)
        coefs = np.zeros((2 * NHC,), np.float32)
        for i in range(NHC):
            h = hs + i
            sl = slice(h * D, (h + 1) * D)
            wqkv[:, i * 128 + 0 : i * 128 + 64] = Wq[sl].T
            wqkv[:, i * 128 + 64 : i * 128 + 128] = Wk[sl].T
            wqkv[:, 384 + i * 64 : 384 + (i + 1) * 64] = Wv[sl].T
            wpt[i] = Wp[:, sl].T
            b01 = float(np.clip(g[h, 1], -2.0, 2.0))
            b10 = float(np.clip(g[h, 0], -2.0, 2.0))
            b11 = float(np.clip(g[h, 0] + g[h, 1] + g[h, 2], -2.0, 2.0))
            coefs[2 * i] = b01            # a2
            coefs[2 * i + 1] = b11 - b10 - b01  # a3
        in_maps.append(
            dict(
                xT=xTb,
                wqkv=wqkv.astype(bf),
                wpt=wpt.astype(bf),
                ropet=ropet,
                mtok=np.ascontiguousarray(m[b].reshape(TCN, 128).T),
                coef=np.tile(coefs[None, :], (128, 1)),
            )
        )

    nc = _get_nc()
    res = run_bass_kernel_spmd(nc, in_maps, list(range(8)))
    global LAST_RESULTS
    LAST_RESULTS = res
    out = np.zeros((B, T, C), np.float32)
    for b in range(B):
        out[b] = res.results[2 * b]["out"] + res.results[2 * b + 1]["out"]
    return out
